# revision 12
# baseline (speedup 1.0000x reference)
"""MoE (DeepSeek-style top-2 routing, E=8 experts) Trainium2 kernel, v3.

Strategy (expert parallelism, per the sharding hint):
  - Host: tiny gate matmul [T,D]@[D,E] + softmax + top-2 (0.02% of FLOPs),
    then dispatch tokens to experts ("all-to-all by topk_idx" done host-side
    while building per-core shards).
  - Device (core e = expert e): yT = W2 @ (silu(W1 @ xT) * (W3 @ xwT))
    where tokens live on the free axis and contraction/feature dims on
    partitions, so no on-device transposes are needed.
  - Host: scatter-add per-expert outputs back to token slots + residual.

v3 vs v1 (the 164 us baseline):
  - NT 272 -> 512 (the PSUM-bank maximum for f32), halving the matmul
    instruction count (1088 -> 544) to amortize per-instruction overhead
    (~15 cycles/instr measured), at the cost of capacity C = 2*512 = 1024 <
    max expert load (1062 for the graded seed): ~92 overflow tokens are
    computed exactly on host in f32 (0.9% of the FLOPs).
  - Input DMA overlaps compute in the single-pass (graded) execution:
    x/xw and the two w2 blocks stream from the otherwise-idle gpsimd
    (Pool) queue while w1/w3 stream from the sync (SP) queue, both in
    consumption order. The PE is gated per input "gate" (one semaphore
    per gate, waited only at its total, since DMA completion order within
    a queue is not guaranteed) and starts after ~2 MB has landed instead
    of after all ~12.4 MB. Output DMAs are issued from the scalar (ACT)
    queue. Measured cold-start overhead over the steady-state pipeline:
    ~25 us with everything on one queue (bench="full"); the graded
    two-queue split is tighter.
  - Outputs are written back as bf16 (half the output traffic; adds
    ~5e-5 to the relative error).
  - Steady-state 138-146 us/rep at sustained 8-core load vs 181 us for
    the v1 geometry measured in the same regime (~20% faster); the PE
    streaming floor at 2.4 GHz is ~119 us, and 8-core DVFS throttling
    (measured 1.25-1.5x vs single-core) accounts for most of the gap.

All three matmul layers run in fp8 (e4m3) with perf_mode=DoubleRow, which
contracts two 128-deep k-planes per instruction (256-contraction) at ~2x
the bf16 rate (measured ~(NT+15) PE cycles per instruction). Scale folding
keeps everything in e4m3 range and recovers true scale exactly at the end:
  - W1,W3,W2 are quantized as 256*W (their entries are ~N(0, 1/sqrt(D))).
  - x is quantized plainly; a second copy xw = 4*combine_weight*x is
    uploaded for the W3 path, which folds the per-token combine weight
    (and the fp8 g headroom factor 4) into a matmul input for free:
    (wv*x)@W3 == wv*(x@W3).
  - silu(ps1/256) via the ACT instruction's scale operand -> t (bf16).
  - g = t * ps3/256 in ONE DVE scalar_tensor_tensor, written as e4m3
    (g = 4*wv*silu(xW1)*(xW3), |g| < ~32 << 240 = e4m3 max).
  - L2 contraction (H=1408 = 11 planes) runs as 5 DoubleRow pairs plus
    one normal fp8 matmul for the odd plane in the same PSUM group.
  - o = psy / 1024 (DVE tensor_scalar_mul, f32) undoes 256*4.

Implementation note: this walrus build allows only ONE semaphore wait per
instruction, which is incompatible with the Tile layer's generated sync.
So the kernel is raw bass: explicit engine programs with standalone
wait_ge instructions and a hand-rolled double-buffering protocol.

`_build_nc(loop=True, bench=...)` wraps the pipeline in per-engine Fori
loops with a runtime rep count (input "nr") for hardware timing: reps are
timed with one executable and per-rep = (wall(R2)-wall(R1))/(R2-R1), so
dispatch/transfer overheads cancel. bench="small" keeps inputs resident
(steady-state pipeline time); bench="full" re-DMAs all inputs every rep
with the same fine-grained gating as the graded single pass (cold-start
proxy). Both use small dram tensors so the axon payload per call is ~2 MB.
The graded path is loop=False, bench=False.
"""

import numpy as np
import ml_dtypes

B, S, D = 2, 2048, 2048
H = 1408
E = 8
T = B * S
P = 128

NT = 512          # token chunk = matmul free dim = max PSUM bank (f32)
NCH = 2           # chunks
C = NT * NCH      # 1024 per-expert token capacity; overflow -> host f32
KD = D // P       # 16 k-planes for the D contraction
KH = H // P       # 11 k-planes for the H contraction
XG = 4            # x/xw input DMA granularity: KD/XG = 4 k-planes per DMA

FP8 = ml_dtypes.float8_e4m3
W_SCALE = 256.0
G_SCALE = 4.0
SILU_SCALE = 1.0 / W_SCALE                 # ps1 -> silu input
G_MUL_SCALE = 1.0 / W_SCALE                # ps3 factor inside the g mul
O_SCALE = 1.0 / (W_SCALE * G_SCALE)        # psy -> true-scale output

# dram shapes for the bench=... builds (values irrelevant, timing only)
BENCH_IN_SHAPES = {
    "xt": (P, XG * C),
    "xwt": (P, XG * C),
    "w1t": (P, KD * P),
    "w3t": (P, KD * P),
    "w2t": (P, KH * P),
}

_CACHE = {}


def _build_nc(loop=False, bench=False, sim_act=False, unroll=1):
    import concourse.bass as bass
    import concourse.mybir as mybir
    from contextlib import ExitStack

    assert bench in (False, "small", "full")
    assert unroll == 1 or bench == "small"
    if bench:
        assert loop

    f32 = mybir.dt.float32
    bf16 = mybir.dt.bfloat16
    fp8 = mybir.dt.float8e4
    i32 = mybir.dt.int32
    ACT_SILU = (mybir.ActivationFunctionType.Sigmoid if sim_act
                else mybir.ActivationFunctionType.Silu)
    MUL = mybir.AluOpType.mult
    DR = mybir.MatmulPerfMode.DoubleRow

    nc = bass.Bass()
    if bench:
        dr = {k: nc.dram_tensor(k, list(v), fp8, kind="ExternalInput").ap()
              for k, v in BENCH_IN_SHAPES.items()}
        xt, xwt = dr["xt"], dr["xwt"]
        w1t, w3t, w2t = dr["w1t"], dr["w3t"], dr["w2t"]
        yt = nc.dram_tensor("yt", [P, NT], bf16, kind="ExternalOutput").ap()
    else:
        xt = nc.dram_tensor("xt", [D, C], fp8, kind="ExternalInput").ap()
        xwt = nc.dram_tensor("xwt", [D, C], fp8, kind="ExternalInput").ap()
        w1t = nc.dram_tensor("w1t", [D, H], fp8, kind="ExternalInput").ap()
        w3t = nc.dram_tensor("w3t", [D, H], fp8, kind="ExternalInput").ap()
        w2t = nc.dram_tensor("w2t", [H, D], fp8, kind="ExternalInput").ap()
        yt = nc.dram_tensor("yt", [D, C], bf16, kind="ExternalOutput").ap()
    if loop:
        nr = nc.dram_tensor("nr", [1, 1], i32, kind="ExternalInput").ap()

    CT = NCH
    NM = CT * KH          # silu / g-mul groups per rep
    NO = CT * KD          # output tiles per rep
    xg = XG if bench else KD   # graded: one 1 MB DMA per (tensor, chunk)
    NXG = KD // xg        # x dma groups per chunk

    with ExitStack() as ctx:
        sb = lambda name, shape, dt: ctx.enter_context(
            nc.sbuf_tensor(name, shape, dt)).ap()
        ps = lambda name, shape: ctx.enter_context(
            nc.psum_tensor(name, shape, f32)).ap()
        sem = lambda name: ctx.enter_context(nc.semaphore(name))

        w1_sb = sb("w1_sb", [P, KD, H], fp8)
        w3_sb = sb("w3_sb", [P, KD, H], fp8)
        w2_sb = sb("w2_sb", [P, KH, D], fp8)
        x_sb = sb("x_sb", [P, KD, C], fp8)
        xw_sb = sb("xw_sb", [P, KD, C], fp8)
        t_sb = [sb(f"t_sb{b}", [P, NT], bf16) for b in range(2)]
        g_sb = [sb(f"g_sb{b}", [P, KH, NT], fp8) for b in range(2)]
        o_sb = [sb(f"o_sb{b}", [P, NT], bf16) for b in range(4)]
        if loop:
            nr_sb = sb("nr_sb", [1, 1], i32)
        ps1 = [ps(f"ps1_{b}", [P, NT]) for b in range(2)]
        ps3 = [ps(f"ps3_{b}", [P, NT]) for b in range(2)]
        psy = [ps(f"psy_{b}", [P, NT]) for b in range(4)]

        dma_nr = sem("dma_nr")
        pe_s = sem("pe_s")
        act_s = sem("act_s")
        dve_s = sem("dve_s")
        s_o = [sem(f"s_o{b}") for b in range(4)]
        if loop:
            done_s = sem("done_s")
            go_s = sem("go_s")

        # ---- input DMA gates. Each gate owns a semaphore and a set of DMAs;
        # consumers wait only for the gate TOTAL (all its DMAs complete), so
        # DMA completion order never matters. x/xw gates are issued from the
        # gpsimd queue, weight gates from the sync queue (HWDGE), both in
        # consumption order, so input transfer overlaps compute in the
        # single-pass (graded) execution.
        W1SPLIT = 4   # w1/w3 gate A covers m 0..3, gate B the rest
        W2SPLIT = 8
        gates = {}    # name -> (sem, total)
        g_x = [sem(f"s_x{c}") for c in range(CT)]
        g_xw = [sem(f"s_xw{c}") for c in range(CT)]
        g_w1 = [sem("s_w1a"), sem("s_w1b")]
        g_w3 = [sem("s_w3a"), sem("s_w3b")]
        g_w2 = [sem("s_w2a"), sem("s_w2b")]
        tot_x = 16 * NXG
        tot_w1 = [16 * W1SPLIT, 16 * (KH - W1SPLIT)]
        if loop:
            tot_w2 = [16 * W2SPLIT, 16 * (KD - W2SPLIT)]
        else:
            tot_w2 = [16, 16]

        def issue_x_dmas(eng):
            for c in range(CT):
                for srct, dst_sb, s in ((xt, x_sb, g_x[c]),
                                        (xwt, xw_sb, g_xw[c])):
                    for g in range(NXG):
                        dst = dst_sb[:, g * xg:(g + 1) * xg,
                                     c * NT:(c + 1) * NT]
                        if bench:
                            src = srct.rearrange(
                                "p (k c) -> p k c",
                                k=xg)[:, :, c * NT:(c + 1) * NT]
                        else:
                            src = srct.rearrange(
                                "(k p) c -> p k c", p=P)[
                                :, g * xg:(g + 1) * xg,
                                c * NT:(c + 1) * NT]
                        eng.dma_start(out=dst, in_=src).then_inc(s, 16)

        def _one_w(eng, srct, dst_sb, m, kdim, s):
            dst = dst_sb[:, :, m * P:(m + 1) * P]
            if bench:
                src = srct.rearrange("p (k c) -> p k c", k=kdim)
            else:
                pat = ("(k p) h -> p k h" if kdim == KD
                       else "(k p) d -> p k d")
                src = srct.rearrange(pat, p=P)[:, :, m * P:(m + 1) * P]
            eng.dma_start(out=dst, in_=src).then_inc(s, 16)

        def issue_w_dmas(eng, with_w2=True):
            for m in range(W1SPLIT):
                _one_w(eng, w1t, w1_sb, m, KD, g_w1[0])
            for m in range(W1SPLIT):
                _one_w(eng, w3t, w3_sb, m, KD, g_w3[0])
            for m in range(W1SPLIT, KH):
                _one_w(eng, w1t, w1_sb, m, KD, g_w1[1])
            for m in range(W1SPLIT, KH):
                _one_w(eng, w3t, w3_sb, m, KD, g_w3[1])
            if with_w2:
                for m2 in range(KD):
                    _one_w(eng, w2t, w2_sb, m2, KH,
                           g_w2[0] if m2 < W2SPLIT else g_w2[1])

        def issue_w2_big(eng):
            # graded only: two big w2 DMAs on this engine's queue, so the
            # sync queue finishes w1/w3 sooner
            for half, s in ((0, g_w2[0]), (1, g_w2[1])):
                lo, hi = half * W2SPLIT * P, (half * W2SPLIT + W2SPLIT) * P
                src = w2t.rearrange("(k p) d -> p k d", p=P)[:, :, lo:hi]
                eng.dma_start(out=w2_sb[:, :, lo:hi], in_=src).then_inc(s, 16)

        # fine-grained PE input gating only when inputs stream during
        # compute: the graded single pass and the bench="full" loop.
        overlap = (not loop) or bench == "full"

        # Semaphore values at each pipeline event (one rep).
        v_ps1, v_ps3, v_psy = [0] * NM, [0] * NM, [0] * NO
        v_silu = [0] * NM
        v_gmul, v_oc = [0] * NM, [0] * NO
        pe_c = act_c = dve_c = 0
        for c in range(CT):
            for m in range(KH):
                i = c * KH + m
                pe_c += 1; v_ps1[i] = pe_c
                pe_c += 1; v_ps3[i] = pe_c
            for m2 in range(KD):
                j = c * KD + m2
                pe_c += 1; v_psy[j] = pe_c
        for i in range(NM):
            act_c += 1; v_silu[i] = act_c
        for c in range(CT):
            for m in range(KH):
                dve_c += 1; v_gmul[c * KH + m] = dve_c
            for m2 in range(KD):
                dve_c += 1; v_oc[c * KD + m2] = dve_c
        pe_total, act_total, dve_total = pe_c, act_c, dve_c

        # unroll>1 (bench="small"): U reps run inside one barrier with
        # compile-time semaphore offsets (u * per-rep count), so the
        # drain/reset cost is paid once per U reps. The SBASE pre-increment
        # (re-applied by gpsimd after each clear) keeps sub-rep-0 lookback
        # thresholds positive; they are trivially satisfied, which is
        # correct because the barrier guarantees the previous superblock
        # fully drained.
        U = unroll
        SBASE = 256 if U > 1 else 0
        so_rep = 16 * (NO // 4)

        from contextlib import contextmanager

        @contextmanager
        def rep_loop(eng):
            """In loop mode: Fori with runtime rep count; else: single pass."""
            if loop:
                r_end = eng.alloc_register(f"nr_{eng.engine.value}")
                eng.reg_load(r_end, nr_sb)
                with eng.Fori(0, r_end) as i:
                    yield i
            else:
                yield None

        def finish_iter(eng, i, self_sem, self_val):
            if loop:
                eng.wait_ge(self_sem, self_val)
                eng.sem_inc(done_s, 1)
                eng.wait_ge(go_s, i + 1)

        n_loopers = 4 if bench == "full" else 3

        with nc.Block() as block:

            @block.sync
            def _(sync):
                if loop:
                    sync.dma_start(out=nr_sb, in_=nr).then_inc(dma_nr, 16)
                if loop:
                    if U > 1:
                        for s in (pe_s, act_s, dve_s, *s_o):
                            sync.sem_inc(s, SBASE)
                    # loop modes: all inputs from the sync queue (gpsimd DMA
                    # issue inside a hw loop desyncs the device)
                    issue_x_dmas(sync)
                issue_w_dmas(sync, with_w2=loop)
                if loop and bench == "full":
                    sync.wait_ge(dma_nr, 16)
                    r_end = sync.alloc_register("sy_nr")
                    sync.reg_load(r_end, nr_sb)
                    rm1 = sync.alloc_register("sy_nrm1")
                    sync.reg_sub(rm1, r_end, 1)
                    with sync.Fori(0, r_end) as it:
                        sync.sem_inc(done_s, 1)
                        sync.wait_ge(go_s, it + 1)
                        with sync.If_cmp(it, rm1, "IS_LT"):
                            issue_x_dmas(sync)
                            issue_w_dmas(sync)

            @block.gpsimd
            def _(gpsimd):
                if not loop:
                    # graded single pass: x/xw then the two big w2 blocks
                    # stream from the gpsimd queue, parallel to w1/w3 on the
                    # sync queue
                    issue_x_dmas(gpsimd)
                    issue_w2_big(gpsimd)
                    return
                gpsimd.wait_ge(dma_nr, 16)
                r_end = gpsimd.alloc_register("gp_nr")
                gpsimd.reg_load(r_end, nr_sb)
                rm1 = gpsimd.alloc_register("gp_nrm1")
                gpsimd.reg_sub(rm1, r_end, 1)
                with gpsimd.Fori(0, r_end) as it:
                    gpsimd.wait_ge(done_s, n_loopers)
                    gpsimd.sem_clear(pe_s)
                    gpsimd.sem_clear(act_s)
                    gpsimd.sem_clear(dve_s)
                    for s in s_o:
                        gpsimd.sem_clear(s)
                    if bench == "full":
                        for s in (*g_x, *g_xw, *g_w1, *g_w3, *g_w2):
                            gpsimd.sem_clear(s)
                    gpsimd.sem_clear(done_s)
                    if U > 1:
                        for s in (pe_s, act_s, dve_s, *s_o):
                            gpsimd.sem_inc(s, SBASE)
                    gpsimd.sem_inc(go_s, 1)


            @block.tensor
            def _(tensor):
                if loop:
                    tensor.wait_ge(dma_nr, 16)
                waited = set()

                def gate_wait(s, val):
                    if (id(s), val) not in waited:
                        waited.add((id(s), val))
                        tensor.wait_ge(s, val)

                if not overlap:
                    for c in range(CT):
                        tensor.wait_ge(g_x[c], tot_x)
                        tensor.wait_ge(g_xw[c], tot_x)
                    for s, tw in zip((*g_w1, *g_w3, *g_w2),
                                     (*tot_w1, *tot_w1, *tot_w2)):
                        tensor.wait_ge(s, tw)
                with rep_loop(tensor) as it:
                    if loop and overlap:
                        waited.clear()
                  # unroll: U sub-reps per barrier, compile-time offsets
                    for u in range(U):
                     for c in range(CT):
                        cols = slice(c * NT, (c + 1) * NT)
                        for m in range(KH):
                            i = c * KH + m
                            msl = slice(m * P, (m + 1) * P)
                            if U > 1:
                                # ps1 slot reuse: silu of sub-rep group G-2
                                # (count u*NM + i - 1), uniform in i.
                                tensor.wait_ge(act_s, SBASE + u * NM + i - 1)
                            elif i >= 2:
                                # ps1 slot reuse: ACT silu of i-2 must be done.
                                tensor.wait_ge(act_s, v_silu[i - 2])
                            if overlap:
                                gate_wait(g_x[c], tot_x)
                                gate_wait(g_w1[0 if m < W1SPLIT else 1],
                                          tot_w1[0 if m < W1SPLIT else 1])
                            for k in range(0, KD, 2):
                                mm = nc.tensor.matmul(
                                    ps1[i % 2], w1_sb[:, k:k + 2, msl],
                                    x_sb[:, k:k + 2, cols],
                                    start=(k == 0), stop=(k == KD - 2),
                                    perf_mode=DR)
                            mm.then_inc(pe_s, 1)
                            if U > 1:
                                # ps3 slot reuse: g-mul of group G-2 (wraps
                                # into the previous sub-rep for i < 2).
                                cg = (v_gmul[i - 2] if i >= 2
                                      else v_gmul[i - 2 + NM] - dve_total)
                                tensor.wait_ge(dve_s,
                                               SBASE + u * dve_total + cg)
                            elif i >= 2:
                                # ps3 slot reuse: DVE g-mul of i-2 must be done.
                                tensor.wait_ge(dve_s, v_gmul[i - 2])
                            if overlap:
                                gate_wait(g_xw[c], tot_x)
                                gate_wait(g_w3[0 if m < W1SPLIT else 1],
                                          tot_w1[0 if m < W1SPLIT else 1])
                            for k in range(0, KD, 2):
                                mm = nc.tensor.matmul(
                                    ps3[i % 2], w3_sb[:, k:k + 2, msl],
                                    xw_sb[:, k:k + 2, cols],
                                    start=(k == 0), stop=(k == KD - 2),
                                    perf_mode=DR)
                            mm.then_inc(pe_s, 1)
                        for m2 in range(KD):
                            j = c * KD + m2
                            m2sl = slice(m2 * P, (m2 + 1) * P)
                            # g planes 0..KH-2 are ready well before the last
                            # one; only the final single matmul reads plane
                            # KH-1, so the group can start while ACT/DVE
                            # finish it.
                            if U > 1:
                                if m2 == 0:
                                    tensor.wait_ge(
                                        dve_s, SBASE + u * dve_total
                                        + v_gmul[c * KH + KH - 2])
                                # psy slot reuse: o-scale of group j-4 (wraps
                                # into the previous sub-rep for j < 4).
                                co = (v_oc[j - 4] if j >= 4
                                      else v_oc[j - 4 + NO] - dve_total)
                                tensor.wait_ge(dve_s,
                                               SBASE + u * dve_total + co)
                            else:
                                need = (v_gmul[c * KH + KH - 2]
                                        if m2 == 0 else 0)
                                if j >= 4:
                                    # psy slot reuse: DVE o-scale of j-4 done.
                                    need = max(need, v_oc[j - 4])
                                if need:
                                    tensor.wait_ge(dve_s, need)
                            if overlap:
                                gate_wait(g_w2[0 if m2 < W2SPLIT else 1],
                                          tot_w2[0 if m2 < W2SPLIT else 1])
                            # 5 DoubleRow pairs (planes 0..9) + one normal
                            # fp8 matmul for the odd plane 10 — no padded
                            # 12th plane to burn cycles on.
                            for k in range(0, KH - 1, 2):
                                nc.tensor.matmul(
                                    psy[j % 4], w2_sb[:, k:k + 2, m2sl],
                                    g_sb[c % 2][:, k:k + 2, :],
                                    start=(k == 0), stop=False,
                                    perf_mode=DR)
                            if m2 == 0:
                                tensor.wait_ge(
                                    dve_s, SBASE + u * dve_total
                                    + v_gmul[c * KH + KH - 1])
                            mm = nc.tensor.matmul(
                                psy[j % 4], w2_sb[:, KH - 1, m2sl],
                                g_sb[c % 2][:, KH - 1, :],
                                start=False, stop=True)
                            mm.then_inc(pe_s, 1)
                    finish_iter(tensor, it, pe_s, SBASE + U * pe_total)

            @block.scalar
            def _(scalar):
                if loop:
                    scalar.wait_ge(dma_nr, 16)
                with rep_loop(scalar) as it:
                  for u in range(U):
                    for c in range(CT):
                        cols = slice(c * NT, (c + 1) * NT)
                        for m in range(KH):
                            i = c * KH + m
                            scalar.wait_ge(pe_s,
                                           SBASE + u * pe_total + v_ps1[i])
                            if U > 1:
                                cg = (v_gmul[i - 2] if i >= 2
                                      else v_gmul[i - 2 + NM] - dve_total)
                                scalar.wait_ge(dve_s,
                                               SBASE + u * dve_total + cg)
                            elif i >= 2:
                                # t slot reuse: DVE g-mul of i-2 must be done.
                                scalar.wait_ge(dve_s, v_gmul[i - 2])
                            nc.scalar.activation(
                                out=t_sb[i % 2], in_=ps1[i % 2],
                                func=ACT_SILU, scale=SILU_SCALE
                            ).then_inc(act_s, 1)
                        # Output DMA issue: all o-scales of chunk c complete
                        # during PE's L2(c), strictly before ps1 of the next
                        # chunk exists, so issuing outs here never delays the
                        # next chunk's silus.
                        for m2 in range(KD):
                            j = c * KD + m2
                            scalar.wait_ge(dve_s,
                                           SBASE + u * dve_total + v_oc[j])
                            scalar.dma_start(
                                out=yt if bench
                                else yt[m2 * P:(m2 + 1) * P, cols],
                                in_=o_sb[j % 4]
                            ).then_inc(s_o[j % 4], 16)
                    for b in range(4):
                        scalar.wait_ge(s_o[b], SBASE + U * so_rep)
                    finish_iter(scalar, it, s_o[3], SBASE + U * so_rep)

            @block.vector
            def _(vector):
                if loop:
                    vector.wait_ge(dma_nr, 16)
                with rep_loop(vector) as it:
                  for u in range(U):
                    for c in range(CT):
                        for m in range(KH):
                            i = c * KH + m
                            vector.wait_ge(act_s,
                                           SBASE + u * NM + v_silu[i])
                            vector.wait_ge(pe_s,
                                           SBASE + u * pe_total + v_ps3[i])
                            nc.vector.scalar_tensor_tensor(
                                out=g_sb[c % 2][:, m, :], in0=ps3[i % 2],
                                scalar=G_MUL_SCALE, in1=t_sb[i % 2],
                                op0=MUL, op1=MUL
                            ).then_inc(dve_s, 1)
                        for m2 in range(KD):
                            j = c * KD + m2
                            vector.wait_ge(pe_s,
                                           SBASE + u * pe_total + v_psy[j])
                            if U > 1:
                                # o slot reuse: out-DMA of the slot's prior
                                # use (previous sub-rep for j < 4).
                                cso = 16 * (j // 4) if j >= 4 else -16
                                vector.wait_ge(s_o[j % 4],
                                               SBASE + u * so_rep + cso)
                            elif j >= 4:
                                # o slot reuse: out-DMA of j-4 must be done.
                                vector.wait_ge(s_o[j % 4], 16 * (j // 4))
                            nc.vector.tensor_scalar_mul(
                                o_sb[j % 4], psy[j % 4], O_SCALE
                            ).then_inc(dve_s, 1)
                    finish_iter(vector, it, dve_s, SBASE + U * dve_total)

    return nc


def _route(x, Wg):
    """Host gate: softmax over expert logits, top-2 selection (f32)."""
    logits = x @ Wg.T                        # [T, E] f32
    m = logits.max(axis=-1, keepdims=True)
    ex = np.exp(logits - m, dtype=np.float32)
    scores = ex / ex.sum(axis=-1, keepdims=True)
    order = np.argsort(-logits, axis=-1, kind="stable")
    top2 = order[:, :2]                      # [T, 2]
    return scores, top2


def kernel(hidden_states, Wg, W1, W3, W2, top_k):
    assert int(top_k) == 2
    x = np.asarray(hidden_states, dtype=np.float32).reshape(T, D)
    Wg = np.asarray(Wg, dtype=np.float32)
    scores, top2 = _route(x, Wg)

    rows = []      # token indices per expert
    wts = []       # combine weights per expert
    for e in range(E):
        sel = np.nonzero((top2 == e).any(axis=1))[0]
        rows.append(sel)
        wts.append(scores[sel, e].astype(np.float32))

    # Capacity overflow: tokens beyond C per expert (~92 for the graded
    # seed at C=1024) are computed on host in f32.
    overflow = []
    for e in range(E):
        if len(rows[e]) > C:
            overflow.append((e, rows[e][C:], wts[e][C:]))
            rows[e] = rows[e][:C]
            wts[e] = wts[e][:C]

    W1 = np.asarray(W1, dtype=np.float32)
    W3 = np.asarray(W3, dtype=np.float32)
    W2 = np.asarray(W2, dtype=np.float32)

    in_maps = []
    for e in range(E):
        n_e = len(rows[e])
        xe = x[rows[e]]                      # [n_e, D]
        xt = np.zeros((D, C), dtype=FP8)
        xt[:, :n_e] = xe.T.astype(FP8)
        xwt = np.zeros((D, C), dtype=FP8)
        xwt[:, :n_e] = (xe * (G_SCALE * wts[e])[:, None]).T.astype(FP8)
        in_maps.append({
            "xt": xt,
            "xwt": xwt,
            "w1t": np.ascontiguousarray(W1[e].T * W_SCALE).astype(FP8),
            "w3t": np.ascontiguousarray(W3[e].T * W_SCALE).astype(FP8),
            "w2t": np.ascontiguousarray(W2[e].T * W_SCALE).astype(FP8),
        })

    if "nc" not in _CACHE:
        _CACHE["nc"] = _build_nc()
    nc = _CACHE["nc"]

    import os
    from concourse.bass_utils import run_bass_kernel_spmd
    trace = os.environ.get("MOE_BASS_TRACE", "") == "1"
    res = run_bass_kernel_spmd(nc, in_maps, core_ids=list(range(E)), trace=trace)
    _CACHE["last_res"] = res
    _CACHE["last_in_maps"] = in_maps

    y = np.zeros((T, D), dtype=np.float32)
    for e in range(E):
        n_e = len(rows[e])
        if n_e:
            y[rows[e]] += res.results[e]["yt"][:, :n_e].T.astype(np.float32)

    for e, sel, w in overflow:
        xe = x[sel]
        h = _silu(xe @ W1[e].T) * (xe @ W3[e].T)
        y[sel] += w[:, None] * (h @ W2[e].T)

    out = y + x
    return out.reshape(B, S, D)


def _silu(v):
    return v / (1.0 + np.exp(-v))


# revision 16
# speedup vs baseline: 1.0905x; 1.0905x over previous
"""MoE (DeepSeek-style top-2 routing, E=8 experts) Trainium2 kernel, v3.

Strategy (expert parallelism, per the sharding hint):
  - Host: tiny gate matmul [T,D]@[D,E] + softmax + top-2 (0.02% of FLOPs),
    then dispatch tokens to experts ("all-to-all by topk_idx" done host-side
    while building per-core shards).
  - Device (core e = expert e): yT = W2 @ (silu(W1 @ xT) * (W3 @ xwT))
    where tokens live on the free axis and contraction/feature dims on
    partitions, so no on-device transposes are needed.
  - Host: scatter-add per-expert outputs back to token slots + residual.

v3 vs v1 (the 164 us baseline):
  - NT 272 -> 512 (the PSUM-bank maximum for f32), halving the matmul
    instruction count (1088 -> 544) to amortize per-instruction overhead
    (~15 cycles/instr measured), at the cost of capacity C = 2*512 = 1024 <
    max expert load (1062 for the graded seed): ~92 overflow tokens are
    computed exactly on host in f32 (0.9% of the FLOPs).
  - Input DMA overlaps compute in the single-pass (graded) execution:
    x/xw and the two w2 blocks stream from the otherwise-idle gpsimd
    (Pool) queue while w1/w3 stream from the sync (SP) queue, both in
    consumption order. The PE is gated per input "gate" (one semaphore
    per gate, waited only at its total, since DMA completion order within
    a queue is not guaranteed) and starts after ~2 MB has landed instead
    of after all ~12.4 MB. Output DMAs are issued from the scalar (ACT)
    queue. Measured cold-start overhead over the steady-state pipeline:
    ~25 us with everything on one queue (bench="full"); the graded
    two-queue split is tighter.
  - Outputs are written back as bf16 (half the output traffic; adds
    ~5e-5 to the relative error).
  - Steady-state 138-146 us/rep at sustained 8-core load vs 181 us for
    the v1 geometry measured in the same regime (~20% faster); the PE
    streaming floor at 2.4 GHz is ~119 us, and 8-core DVFS throttling
    (measured 1.25-1.5x vs single-core) accounts for most of the gap.

All three matmul layers run in fp8 (e4m3) with perf_mode=DoubleRow, which
contracts two 128-deep k-planes per instruction (256-contraction) at ~2x
the bf16 rate (measured ~(NT+15) PE cycles per instruction). Scale folding
keeps everything in e4m3 range and recovers true scale exactly at the end:
  - W1,W3,W2 are quantized as 256*W (their entries are ~N(0, 1/sqrt(D))).
  - x is quantized plainly; a second copy xw = 4*combine_weight*x is
    uploaded for the W3 path, which folds the per-token combine weight
    (and the fp8 g headroom factor 4) into a matmul input for free:
    (wv*x)@W3 == wv*(x@W3).
  - silu(ps1/256) via the ACT instruction's scale operand -> t (bf16).
  - g = t * ps3/256 in ONE DVE scalar_tensor_tensor, written as e4m3
    (g = 4*wv*silu(xW1)*(xW3), |g| < ~32 << 240 = e4m3 max).
  - L2 contraction (H=1408 = 11 planes) runs as 5 DoubleRow pairs plus
    one normal fp8 matmul for the odd plane in the same PSUM group.
  - o = psy / 1024 (DVE tensor_scalar_mul, f32) undoes 256*4.

Implementation note: this walrus build allows only ONE semaphore wait per
instruction, which is incompatible with the Tile layer's generated sync.
So the kernel is raw bass: explicit engine programs with standalone
wait_ge instructions and a hand-rolled double-buffering protocol.

`_build_nc(loop=True, bench=...)` wraps the pipeline in per-engine Fori
loops with a runtime rep count (input "nr") for hardware timing: reps are
timed with one executable and per-rep = (wall(R2)-wall(R1))/(R2-R1), so
dispatch/transfer overheads cancel. bench="small" keeps inputs resident
(steady-state pipeline time); bench="full" re-DMAs all inputs every rep
with the same fine-grained gating as the graded single pass (cold-start
proxy). Both use small dram tensors so the axon payload per call is ~2 MB.
The graded path is loop=False, bench=False.
"""

import numpy as np
import ml_dtypes

B, S, D = 2, 2048, 2048
H = 1408
E = 8
T = B * S
P = 128

NT = 512          # token chunk = matmul free dim = max PSUM bank (f32)
NCH = 2           # chunks
C = NT * NCH      # 1024 per-expert token capacity; overflow -> host f32
KD = D // P       # 16 k-planes for the D contraction
KH = H // P       # 11 k-planes for the H contraction
XG = 4            # x/xw input DMA granularity: KD/XG = 4 k-planes per DMA

FP8 = ml_dtypes.float8_e4m3
W_SCALE = 256.0
G_SCALE = 4.0
SILU_SCALE = 1.0 / W_SCALE                 # ps1 -> silu input
G_MUL_SCALE = 1.0 / W_SCALE                # ps3 factor inside the g mul
O_SCALE = 1.0 / (W_SCALE * G_SCALE)        # psy -> true-scale output

# dram shapes for the bench=... builds (values irrelevant, timing only)
BENCH_IN_SHAPES = {
    "xt": (P, XG * C),
    "xwt": (P, XG * C),
    "w1t": (P, KD * P),
    "w3t": (P, KD * P),
    "w2t": (P, KH * P),
}

_CACHE = {}


def _build_nc(loop=False, bench=False, sim_act=False, unroll=1):
    import concourse.bass as bass
    import concourse.mybir as mybir
    from contextlib import ExitStack

    assert bench in (False, "small", "full")
    assert unroll == 1 or bench == "small"
    if bench:
        assert loop

    f32 = mybir.dt.float32
    bf16 = mybir.dt.bfloat16
    fp8 = mybir.dt.float8e4
    i32 = mybir.dt.int32
    ACT_SILU = (mybir.ActivationFunctionType.Sigmoid if sim_act
                else mybir.ActivationFunctionType.Silu)
    MUL = mybir.AluOpType.mult
    DR = mybir.MatmulPerfMode.DoubleRow

    nc = bass.Bass()
    if bench:
        dr = {k: nc.dram_tensor(k, list(v), fp8, kind="ExternalInput").ap()
              for k, v in BENCH_IN_SHAPES.items()}
        xt, xwt = dr["xt"], dr["xwt"]
        w1t, w3t, w2t = dr["w1t"], dr["w3t"], dr["w2t"]
        yt = nc.dram_tensor("yt", [P, NT], bf16, kind="ExternalOutput").ap()
    else:
        xt = nc.dram_tensor("xt", [D, C], fp8, kind="ExternalInput").ap()
        xwt = nc.dram_tensor("xwt", [D, C], fp8, kind="ExternalInput").ap()
        w1t = nc.dram_tensor("w1t", [D, H], fp8, kind="ExternalInput").ap()
        w3t = nc.dram_tensor("w3t", [D, H], fp8, kind="ExternalInput").ap()
        w2t = nc.dram_tensor("w2t", [H, D], fp8, kind="ExternalInput").ap()
        yt = nc.dram_tensor("yt", [D, C], bf16, kind="ExternalOutput").ap()
    if loop:
        nr = nc.dram_tensor("nr", [1, 1], i32, kind="ExternalInput").ap()

    CT = NCH
    NM = CT * KH          # silu / g-mul groups per rep
    NO = CT * KD          # output tiles per rep
    xg = XG if bench else KD   # graded: one 1 MB DMA per (tensor, chunk)
    NXG = KD // xg        # x dma groups per chunk

    with ExitStack() as ctx:
        sb = lambda name, shape, dt: ctx.enter_context(
            nc.sbuf_tensor(name, shape, dt)).ap()
        ps = lambda name, shape: ctx.enter_context(
            nc.psum_tensor(name, shape, f32)).ap()
        sem = lambda name: ctx.enter_context(nc.semaphore(name))

        w1_sb = sb("w1_sb", [P, KD, H], fp8)
        w3_sb = sb("w3_sb", [P, KD, H], fp8)
        w2_sb = sb("w2_sb", [P, KH, D], fp8)
        x_sb = sb("x_sb", [P, KD, C], fp8)
        xw_sb = sb("xw_sb", [P, KD, C], fp8)
        t_sb = [sb(f"t_sb{b}", [P, NT], bf16) for b in range(2)]
        g_sb = [sb(f"g_sb{b}", [P, KH, NT], fp8) for b in range(2)]
        o_sb = [sb(f"o_sb{b}", [P, NT], bf16) for b in range(4)]
        if loop:
            nr_sb = sb("nr_sb", [1, 1], i32)
        ps1 = [ps(f"ps1_{b}", [P, NT]) for b in range(2)]
        ps3 = [ps(f"ps3_{b}", [P, NT]) for b in range(2)]
        psy = [ps(f"psy_{b}", [P, NT]) for b in range(4)]

        dma_nr = sem("dma_nr")
        pe_s = sem("pe_s")
        act_s = sem("act_s")
        dve_s = sem("dve_s")
        s_o = [sem(f"s_o{b}") for b in range(4)]
        if loop:
            done_s = sem("done_s")
            go_s = sem("go_s")

        # ---- input DMA gates. Each gate owns a semaphore and a set of DMAs;
        # consumers wait only for the gate TOTAL (all its DMAs complete), so
        # DMA completion order never matters. x/xw gates are issued from the
        # gpsimd queue, weight gates from the sync queue (HWDGE), both in
        # consumption order, so input transfer overlaps compute in the
        # single-pass (graded) execution.
        W1SPLIT = 4   # w1/w3 gate A covers m 0..3, gate B the rest
        W2SPLIT = 8
        gates = {}    # name -> (sem, total)
        g_x = [sem(f"s_x{c}") for c in range(CT)]
        g_xw = [sem(f"s_xw{c}") for c in range(CT)]
        g_w1 = [sem("s_w1a"), sem("s_w1b")]
        g_w3 = [sem("s_w3a"), sem("s_w3b")]
        g_w2 = [sem("s_w2a"), sem("s_w2b")]
        tot_x = 16 * NXG
        if bench:
            tot_w1 = [16 * W1SPLIT, 16 * (KH - W1SPLIT)]
        else:
            tot_w1 = [16, 16]
        if loop:
            tot_w2 = [16 * W2SPLIT, 16 * (KD - W2SPLIT)]
        else:
            tot_w2 = [16, 16]

        def issue_x_dmas(eng):
            for c in range(CT):
                for srct, dst_sb, s in ((xt, x_sb, g_x[c]),
                                        (xwt, xw_sb, g_xw[c])):
                    for g in range(NXG):
                        dst = dst_sb[:, g * xg:(g + 1) * xg,
                                     c * NT:(c + 1) * NT]
                        if bench:
                            src = srct.rearrange(
                                "p (k c) -> p k c",
                                k=xg)[:, :, c * NT:(c + 1) * NT]
                        else:
                            src = srct.rearrange(
                                "(k p) c -> p k c", p=P)[
                                :, g * xg:(g + 1) * xg,
                                c * NT:(c + 1) * NT]
                        eng.dma_start(out=dst, in_=src).then_inc(s, 16)

        def _one_w(eng, srct, dst_sb, m, kdim, s):
            dst = dst_sb[:, :, m * P:(m + 1) * P]
            if bench:
                src = srct.rearrange("p (k c) -> p k c", k=kdim)
            else:
                pat = ("(k p) h -> p k h" if kdim == KD
                       else "(k p) d -> p k d")
                src = srct.rearrange(pat, p=P)[:, :, m * P:(m + 1) * P]
            eng.dma_start(out=dst, in_=src).then_inc(s, 16)

        def issue_w_dmas(eng, with_w2=True):
            if not bench:
                # graded: one merged DMA per gate — column-block slices have
                # 512-896 B contiguous runs vs 128 B for per-m slices, so
                # descriptor count drops 4-7x and HBM burst efficiency rises
                mid = W1SPLIT * P
                for srct, dst_sb, ga, gb in ((w1t, w1_sb, g_w1[0], g_w1[1]),
                                             (w3t, w3_sb, g_w3[0], g_w3[1])):
                    src = srct.rearrange("(k p) h -> p k h", p=P)
                    eng.dma_start(out=dst_sb[:, :, :mid],
                                  in_=src[:, :, :mid]).then_inc(ga, 16)
                    eng.dma_start(out=dst_sb[:, :, mid:],
                                  in_=src[:, :, mid:]).then_inc(gb, 16)
            else:
                for m in range(W1SPLIT):
                    _one_w(eng, w1t, w1_sb, m, KD, g_w1[0])
                for m in range(W1SPLIT):
                    _one_w(eng, w3t, w3_sb, m, KD, g_w3[0])
                for m in range(W1SPLIT, KH):
                    _one_w(eng, w1t, w1_sb, m, KD, g_w1[1])
                for m in range(W1SPLIT, KH):
                    _one_w(eng, w3t, w3_sb, m, KD, g_w3[1])
            if with_w2:
                for m2 in range(KD):
                    _one_w(eng, w2t, w2_sb, m2, KH,
                           g_w2[0] if m2 < W2SPLIT else g_w2[1])

        def issue_w2_big(eng):
            # graded only: two big w2 DMAs on this engine's queue, so the
            # sync queue finishes w1/w3 sooner
            for half, s in ((0, g_w2[0]), (1, g_w2[1])):
                lo, hi = half * W2SPLIT * P, (half * W2SPLIT + W2SPLIT) * P
                src = w2t.rearrange("(k p) d -> p k d", p=P)[:, :, lo:hi]
                eng.dma_start(out=w2_sb[:, :, lo:hi], in_=src).then_inc(s, 16)

        # fine-grained PE input gating only when inputs stream during
        # compute: the graded single pass and the bench="full" loop.
        overlap = (not loop) or bench == "full"

        # Semaphore values at each pipeline event (one rep).
        v_ps1, v_ps3, v_psy = [0] * NM, [0] * NM, [0] * NO
        v_silu = [0] * NM
        v_gmul, v_oc = [0] * NM, [0] * NO
        pe_c = act_c = dve_c = 0
        for c in range(CT):
            for m in range(KH):
                i = c * KH + m
                pe_c += 1; v_ps1[i] = pe_c
                pe_c += 1; v_ps3[i] = pe_c
            for m2 in range(KD):
                j = c * KD + m2
                pe_c += 1; v_psy[j] = pe_c
        for i in range(NM):
            act_c += 1; v_silu[i] = act_c
        for c in range(CT):
            for m in range(KH):
                dve_c += 1; v_gmul[c * KH + m] = dve_c
            for m2 in range(KD):
                dve_c += 1; v_oc[c * KD + m2] = dve_c
        pe_total, act_total, dve_total = pe_c, act_c, dve_c

        # unroll>1 (bench="small"): U reps run inside one barrier with
        # compile-time semaphore offsets (u * per-rep count), so the
        # drain/reset cost is paid once per U reps. The SBASE pre-increment
        # (re-applied by gpsimd after each clear) keeps sub-rep-0 lookback
        # thresholds positive; they are trivially satisfied, which is
        # correct because the barrier guarantees the previous superblock
        # fully drained.
        U = unroll
        SBASE = 256 if U > 1 else 0
        so_rep = 16 * (NO // 4)

        from contextlib import contextmanager

        @contextmanager
        def rep_loop(eng):
            """In loop mode: Fori with runtime rep count; else: single pass."""
            if loop:
                r_end = eng.alloc_register(f"nr_{eng.engine.value}")
                eng.reg_load(r_end, nr_sb)
                with eng.Fori(0, r_end) as i:
                    yield i
            else:
                yield None

        def finish_iter(eng, i, self_sem, self_val):
            if loop:
                eng.wait_ge(self_sem, self_val)
                eng.sem_inc(done_s, 1)
                eng.wait_ge(go_s, i + 1)

        n_loopers = 4 if bench == "full" else 3

        with nc.Block() as block:

            @block.sync
            def _(sync):
                if loop:
                    sync.dma_start(out=nr_sb, in_=nr).then_inc(dma_nr, 16)
                if loop:
                    if U > 1:
                        for s in (pe_s, act_s, dve_s, *s_o):
                            sync.sem_inc(s, SBASE)
                    # loop modes: all inputs from the sync queue (gpsimd DMA
                    # issue inside a hw loop desyncs the device)
                    issue_x_dmas(sync)
                issue_w_dmas(sync, with_w2=loop)
                if loop and bench == "full":
                    sync.wait_ge(dma_nr, 16)
                    r_end = sync.alloc_register("sy_nr")
                    sync.reg_load(r_end, nr_sb)
                    rm1 = sync.alloc_register("sy_nrm1")
                    sync.reg_sub(rm1, r_end, 1)
                    with sync.Fori(0, r_end) as it:
                        sync.sem_inc(done_s, 1)
                        sync.wait_ge(go_s, it + 1)
                        with sync.If_cmp(it, rm1, "IS_LT"):
                            issue_x_dmas(sync)
                            issue_w_dmas(sync)

            @block.gpsimd
            def _(gpsimd):
                if not loop:
                    # graded single pass: x/xw then the two big w2 blocks
                    # stream from the gpsimd queue, parallel to w1/w3 on the
                    # sync queue
                    issue_x_dmas(gpsimd)
                    issue_w2_big(gpsimd)
                    return
                gpsimd.wait_ge(dma_nr, 16)
                r_end = gpsimd.alloc_register("gp_nr")
                gpsimd.reg_load(r_end, nr_sb)
                rm1 = gpsimd.alloc_register("gp_nrm1")
                gpsimd.reg_sub(rm1, r_end, 1)
                with gpsimd.Fori(0, r_end) as it:
                    gpsimd.wait_ge(done_s, n_loopers)
                    gpsimd.sem_clear(pe_s)
                    gpsimd.sem_clear(act_s)
                    gpsimd.sem_clear(dve_s)
                    for s in s_o:
                        gpsimd.sem_clear(s)
                    if bench == "full":
                        for s in (*g_x, *g_xw, *g_w1, *g_w3, *g_w2):
                            gpsimd.sem_clear(s)
                    gpsimd.sem_clear(done_s)
                    if U > 1:
                        for s in (pe_s, act_s, dve_s, *s_o):
                            gpsimd.sem_inc(s, SBASE)
                    gpsimd.sem_inc(go_s, 1)


            @block.tensor
            def _(tensor):
                if loop:
                    tensor.wait_ge(dma_nr, 16)
                waited = set()

                def gate_wait(s, val):
                    if (id(s), val) not in waited:
                        waited.add((id(s), val))
                        tensor.wait_ge(s, val)

                if not overlap:
                    for c in range(CT):
                        tensor.wait_ge(g_x[c], tot_x)
                        tensor.wait_ge(g_xw[c], tot_x)
                    for s, tw in zip((*g_w1, *g_w3, *g_w2),
                                     (*tot_w1, *tot_w1, *tot_w2)):
                        tensor.wait_ge(s, tw)
                with rep_loop(tensor) as it:
                    if loop and overlap:
                        waited.clear()
                  # unroll: U sub-reps per barrier, compile-time offsets
                    for u in range(U):
                     for c in range(CT):
                        cols = slice(c * NT, (c + 1) * NT)
                        for m in range(KH):
                            i = c * KH + m
                            msl = slice(m * P, (m + 1) * P)
                            if U > 1:
                                # ps1 slot reuse: silu of sub-rep group G-2
                                # (count u*NM + i - 1), uniform in i.
                                tensor.wait_ge(act_s, SBASE + u * NM + i - 1)
                            elif i >= 2:
                                # ps1 slot reuse: ACT silu of i-2 must be done.
                                tensor.wait_ge(act_s, v_silu[i - 2])
                            if overlap:
                                gate_wait(g_x[c], tot_x)
                                gate_wait(g_w1[0 if m < W1SPLIT else 1],
                                          tot_w1[0 if m < W1SPLIT else 1])
                            for k in range(0, KD, 2):
                                mm = nc.tensor.matmul(
                                    ps1[i % 2], w1_sb[:, k:k + 2, msl],
                                    x_sb[:, k:k + 2, cols],
                                    start=(k == 0), stop=(k == KD - 2),
                                    perf_mode=DR)
                            mm.then_inc(pe_s, 1)
                            if U > 1:
                                # ps3 slot reuse: g-mul of group G-2 (wraps
                                # into the previous sub-rep for i < 2).
                                cg = (v_gmul[i - 2] if i >= 2
                                      else v_gmul[i - 2 + NM] - dve_total)
                                tensor.wait_ge(dve_s,
                                               SBASE + u * dve_total + cg)
                            elif i >= 2:
                                # ps3 slot reuse: DVE g-mul of i-2 must be done.
                                tensor.wait_ge(dve_s, v_gmul[i - 2])
                            if overlap:
                                gate_wait(g_xw[c], tot_x)
                                gate_wait(g_w3[0 if m < W1SPLIT else 1],
                                          tot_w1[0 if m < W1SPLIT else 1])
                            for k in range(0, KD, 2):
                                mm = nc.tensor.matmul(
                                    ps3[i % 2], w3_sb[:, k:k + 2, msl],
                                    xw_sb[:, k:k + 2, cols],
                                    start=(k == 0), stop=(k == KD - 2),
                                    perf_mode=DR)
                            mm.then_inc(pe_s, 1)
                        for m2 in range(KD):
                            j = c * KD + m2
                            m2sl = slice(m2 * P, (m2 + 1) * P)
                            # g planes 0..KH-2 are ready well before the last
                            # one; only the final single matmul reads plane
                            # KH-1, so the group can start while ACT/DVE
                            # finish it.
                            if U > 1:
                                if m2 == 0:
                                    tensor.wait_ge(
                                        dve_s, SBASE + u * dve_total
                                        + v_gmul[c * KH + KH - 2])
                                # psy slot reuse: o-scale of group j-4 (wraps
                                # into the previous sub-rep for j < 4).
                                co = (v_oc[j - 4] if j >= 4
                                      else v_oc[j - 4 + NO] - dve_total)
                                tensor.wait_ge(dve_s,
                                               SBASE + u * dve_total + co)
                            else:
                                need = (v_gmul[c * KH + KH - 2]
                                        if m2 == 0 else 0)
                                if j >= 4:
                                    # psy slot reuse: DVE o-scale of j-4 done.
                                    need = max(need, v_oc[j - 4])
                                if need:
                                    tensor.wait_ge(dve_s, need)
                            if overlap:
                                gate_wait(g_w2[0 if m2 < W2SPLIT else 1],
                                          tot_w2[0 if m2 < W2SPLIT else 1])
                            # 5 DoubleRow pairs (planes 0..9) + one normal
                            # fp8 matmul for the odd plane 10 — no padded
                            # 12th plane to burn cycles on.
                            for k in range(0, KH - 1, 2):
                                nc.tensor.matmul(
                                    psy[j % 4], w2_sb[:, k:k + 2, m2sl],
                                    g_sb[c % 2][:, k:k + 2, :],
                                    start=(k == 0), stop=False,
                                    perf_mode=DR)
                            if m2 == 0:
                                tensor.wait_ge(
                                    dve_s, SBASE + u * dve_total
                                    + v_gmul[c * KH + KH - 1])
                            mm = nc.tensor.matmul(
                                psy[j % 4], w2_sb[:, KH - 1, m2sl],
                                g_sb[c % 2][:, KH - 1, :],
                                start=False, stop=True)
                            mm.then_inc(pe_s, 1)
                    finish_iter(tensor, it, pe_s, SBASE + U * pe_total)

            @block.scalar
            def _(scalar):
                if loop:
                    scalar.wait_ge(dma_nr, 16)
                with rep_loop(scalar) as it:
                  for u in range(U):
                    for c in range(CT):
                        cols = slice(c * NT, (c + 1) * NT)
                        for m in range(KH):
                            i = c * KH + m
                            scalar.wait_ge(pe_s,
                                           SBASE + u * pe_total + v_ps1[i])
                            if U > 1:
                                cg = (v_gmul[i - 2] if i >= 2
                                      else v_gmul[i - 2 + NM] - dve_total)
                                scalar.wait_ge(dve_s,
                                               SBASE + u * dve_total + cg)
                            elif i >= 2:
                                # t slot reuse: DVE g-mul of i-2 must be done.
                                scalar.wait_ge(dve_s, v_gmul[i - 2])
                            nc.scalar.activation(
                                out=t_sb[i % 2], in_=ps1[i % 2],
                                func=ACT_SILU, scale=SILU_SCALE
                            ).then_inc(act_s, 1)
                        # Output DMA issue: all o-scales of chunk c complete
                        # during PE's L2(c), strictly before ps1 of the next
                        # chunk exists, so issuing outs here never delays the
                        # next chunk's silus.
                        for m2 in range(KD):
                            j = c * KD + m2
                            scalar.wait_ge(dve_s,
                                           SBASE + u * dve_total + v_oc[j])
                            scalar.dma_start(
                                out=yt if bench
                                else yt[m2 * P:(m2 + 1) * P, cols],
                                in_=o_sb[j % 4]
                            ).then_inc(s_o[j % 4], 16)
                  for b in range(4):
                    scalar.wait_ge(s_o[b], SBASE + U * so_rep)
                  finish_iter(scalar, it, s_o[3], SBASE + U * so_rep)

            @block.vector
            def _(vector):
                if loop:
                    vector.wait_ge(dma_nr, 16)
                with rep_loop(vector) as it:
                  for u in range(U):
                    for c in range(CT):
                        for m in range(KH):
                            i = c * KH + m
                            vector.wait_ge(act_s,
                                           SBASE + u * NM + v_silu[i])
                            vector.wait_ge(pe_s,
                                           SBASE + u * pe_total + v_ps3[i])
                            nc.vector.scalar_tensor_tensor(
                                out=g_sb[c % 2][:, m, :], in0=ps3[i % 2],
                                scalar=G_MUL_SCALE, in1=t_sb[i % 2],
                                op0=MUL, op1=MUL
                            ).then_inc(dve_s, 1)
                        for m2 in range(KD):
                            j = c * KD + m2
                            vector.wait_ge(pe_s,
                                           SBASE + u * pe_total + v_psy[j])
                            if U > 1:
                                # o slot reuse: out-DMA of the slot's prior
                                # use (previous sub-rep for j < 4).
                                cso = 16 * (j // 4) if j >= 4 else -16
                                vector.wait_ge(s_o[j % 4],
                                               SBASE + u * so_rep + cso)
                            elif j >= 4:
                                # o slot reuse: out-DMA of j-4 must be done.
                                vector.wait_ge(s_o[j % 4], 16 * (j // 4))
                            nc.vector.tensor_scalar_mul(
                                o_sb[j % 4], psy[j % 4], O_SCALE
                            ).then_inc(dve_s, 1)
                  finish_iter(vector, it, dve_s, SBASE + U * dve_total)

    return nc


def _route(x, Wg):
    """Host gate: softmax over expert logits, top-2 selection (f32)."""
    logits = x @ Wg.T                        # [T, E] f32
    m = logits.max(axis=-1, keepdims=True)
    ex = np.exp(logits - m, dtype=np.float32)
    scores = ex / ex.sum(axis=-1, keepdims=True)
    order = np.argsort(-logits, axis=-1, kind="stable")
    top2 = order[:, :2]                      # [T, 2]
    return scores, top2


def kernel(hidden_states, Wg, W1, W3, W2, top_k):
    assert int(top_k) == 2
    x = np.asarray(hidden_states, dtype=np.float32).reshape(T, D)
    Wg = np.asarray(Wg, dtype=np.float32)
    scores, top2 = _route(x, Wg)

    rows = []      # token indices per expert
    wts = []       # combine weights per expert
    for e in range(E):
        sel = np.nonzero((top2 == e).any(axis=1))[0]
        rows.append(sel)
        wts.append(scores[sel, e].astype(np.float32))

    # Capacity overflow: tokens beyond C per expert (~92 for the graded
    # seed at C=1024) are computed on host in f32.
    overflow = []
    for e in range(E):
        if len(rows[e]) > C:
            overflow.append((e, rows[e][C:], wts[e][C:]))
            rows[e] = rows[e][:C]
            wts[e] = wts[e][:C]

    W1 = np.asarray(W1, dtype=np.float32)
    W3 = np.asarray(W3, dtype=np.float32)
    W2 = np.asarray(W2, dtype=np.float32)

    in_maps = []
    for e in range(E):
        n_e = len(rows[e])
        xe = x[rows[e]]                      # [n_e, D]
        xt = np.zeros((D, C), dtype=FP8)
        xt[:, :n_e] = xe.T.astype(FP8)
        xwt = np.zeros((D, C), dtype=FP8)
        xwt[:, :n_e] = (xe * (G_SCALE * wts[e])[:, None]).T.astype(FP8)
        in_maps.append({
            "xt": xt,
            "xwt": xwt,
            "w1t": np.ascontiguousarray(W1[e].T * W_SCALE).astype(FP8),
            "w3t": np.ascontiguousarray(W3[e].T * W_SCALE).astype(FP8),
            "w2t": np.ascontiguousarray(W2[e].T * W_SCALE).astype(FP8),
        })

    if "nc" not in _CACHE:
        _CACHE["nc"] = _build_nc()
    nc = _CACHE["nc"]

    import os
    from concourse.bass_utils import run_bass_kernel_spmd
    trace = os.environ.get("MOE_BASS_TRACE", "") == "1"
    res = run_bass_kernel_spmd(nc, in_maps, core_ids=list(range(E)), trace=trace)
    _CACHE["last_res"] = res
    _CACHE["last_in_maps"] = in_maps

    y = np.zeros((T, D), dtype=np.float32)
    for e in range(E):
        n_e = len(rows[e])
        if n_e:
            y[rows[e]] += res.results[e]["yt"][:, :n_e].T.astype(np.float32)

    for e, sel, w in overflow:
        xe = x[sel]
        h = _silu(xe @ W1[e].T) * (xe @ W3[e].T)
        y[sel] += w[:, None] * (h @ W2[e].T)

    out = y + x
    return out.reshape(B, S, D)


def _silu(v):
    return v / (1.0 + np.exp(-v))
